# revision 2
# baseline (speedup 1.0000x reference)
"""Trainium2 kernel for nn_NodeEdgeProjection (gnn_message_passing).

Reference computes out = x[:, idx, :] with idx = permutations(range(128), 2)[:, 0]
= [0]*127, [1]*127, ..., i.e. idx[e] = e // 127. So the output is each node row
repeated 127 times along the edge axis — a pure broadcast of [B, N, F] to
[B, N*(N-1), F]. Memory-bound: ~533 MB of output writes.

Strategy: pure data parallel over the batch dim (16 batches per core, 8 cores).
Per batch, the node rows live one-per-partition in SBUF; a small DVE doubling
chain builds a 16-repeat replica block (4 KB/partition), and two HWDGE DMAs
write the 127 repeats to DRAM using a stride-0 (broadcast) source dimension so
the SBUF read side streams 4 KB contiguous descriptors.
"""

import numpy as np

B, N, F = 128, 128, 64
NCORES = 8
BPC = B // NCORES   # batches per core: 16
R = N - 1           # repeats per node: 127
W = 16              # repeats materialized in SBUF
K = R // W          # 7 broadcast blocks
REM = R % W         # 15 remaining repeats

_CACHE = {}


def _build_nc(n_reps: int = 1):
    # n_reps repeats the whole body (same output written each time) — used
    # only by the local timing harness to measure steady-state HW time.
    import concourse.bacc as bacc
    import concourse.mybir as mybir
    import concourse.tile as tile

    nc = bacc.Bacc("TRN2", target_bir_lowering=False, debug=False)
    x = nc.dram_tensor("x", [BPC, N, F], mybir.dt.float32, kind="ExternalInput")
    y = nc.dram_tensor("y", [BPC, N * R, F], mybir.dt.float32, kind="ExternalOutput")

    with tile.TileContext(nc) as tc:
        with (
            tc.tile_pool(name="inp", bufs=1) as inpool,
            tc.tile_pool(name="rep", bufs=4) as reppool,
        ):
            in_t = inpool.tile([N, BPC * F], mybir.dt.float32)
            nc.sync.dma_start(
                in_t[:].rearrange("n (b f) -> n b f", b=BPC),
                x.ap().rearrange("b n f -> n b f"),
            )
            for _ in range(n_reps):
                for b in range(BPC):
                    rep = reppool.tile([N, W * F], mybir.dt.float32)
                    nc.vector.tensor_copy(rep[:, 0:F], in_t[:, b * F : (b + 1) * F])
                    w = 1
                    while w < W:
                        nc.vector.tensor_copy(
                            rep[:, w * F : 2 * w * F], rep[:, 0 : w * F]
                        )
                        w *= 2
                    y3 = y.ap()[b].rearrange("(n r) f -> n r f", r=R)
                    # repeats [0, K*W): K copies of the whole replica block
                    nc.sync.dma_start(
                        y3[:, 0 : K * W, :],
                        rep[:].unsqueeze(1).to_broadcast((N, K, W * F)),
                    )
                    # repeats [K*W, R): first REM repeats of the replica block
                    nc.scalar.dma_start(y3[:, K * W : R, :], rep[:, 0 : REM * F])
    nc.compile()
    return nc


def kernel(x: np.ndarray) -> np.ndarray:
    from concourse.bass_utils import run_bass_kernel_spmd

    x = np.ascontiguousarray(np.asarray(x, dtype=np.float32))
    assert x.shape == (B, N, F), x.shape

    if "nc" not in _CACHE:
        _CACHE["nc"] = _build_nc()
    nc = _CACHE["nc"]

    in_maps = [{"x": x[c * BPC : (c + 1) * BPC]} for c in range(NCORES)]
    res = run_bass_kernel_spmd(nc, in_maps, list(range(NCORES)))
    out = np.concatenate([res.results[c]["y"] for c in range(NCORES)], axis=0)
    return out


# revision 3
# speedup vs baseline: 74.0336x; 74.0336x over previous
"""Trainium2 kernel for nn_NodeEdgeProjection (gnn_message_passing).

Reference computes out = x[:, idx, :] with idx = permutations(range(128), 2)[:, 0]
= [0]*127, [1]*127, ..., i.e. idx[e] = e // 127. So the output is each node row
repeated 127 times along the edge axis — a pure broadcast of [B, N, F] to
[B, N*(N-1), F]. Memory-bound: ~533 MB of output writes.

Sharding: pure data parallel over the batch dim (16 batches per core, 8 cores).

Per-core kernel: nodes live one-per-partition in SBUF. For each pair of
batches, a DVE doubling chain materializes all 127 repeats in a pair tile
(2 x 32.5 KB per partition), then two fully-contiguous 4.16 MB HWDGE DMAs
(alternating the SP and ACT rings) stream the pair to DRAM. Measured on HW
(marginal over in-NEFF repeats): ~125-140 us/core, DMA-bound at ~535 GB/s —
on par with a pure-DMA lower bound of the same traffic. A stride-0
(broadcast-source) DMA variant was 90x slower on HW despite the cost model
liking it; replicate-in-SBUF + contiguous DMA is the fast path.
"""

import numpy as np

B, N, F = 128, 128, 64
NCORES = 8
BPC = B // NCORES   # batches per core: 16
R = N - 1           # repeats per node: 127

_CACHE = {}


def _build_nc(n_reps: int = 1):
    # n_reps repeats the whole body (same output written each time) — used
    # only by the local timing harness to measure steady-state HW time.
    import concourse.bacc as bacc
    import concourse.mybir as mybir
    import concourse.tile as tile

    fp32 = mybir.dt.float32
    nc = bacc.Bacc("TRN2", target_bir_lowering=False, debug=False)
    x = nc.dram_tensor("x", [BPC, N, F], fp32, kind="ExternalInput")
    y = nc.dram_tensor("y", [BPC, N * R, F], fp32, kind="ExternalOutput")

    with tile.TileContext(nc) as tc:
        with (
            tc.tile_pool(name="inp", bufs=2) as inpool,
            tc.tile_pool(name="rep", bufs=2) as reppool,
        ):
            for _ in range(n_reps):
                for p in range(BPC // 2):
                    # load the pair's two batches: x[b, n, f] -> in_t[n, (b f)]
                    in_t = inpool.tile([N, 2 * F], fp32)
                    nc.sync.dma_start(
                        in_t[:].rearrange("n (b f) -> n b f", b=2),
                        x.ap()[2 * p : 2 * p + 2].rearrange("b n f -> n b f"),
                    )
                    rep = reppool.tile([N, 2 * R * F], fp32)
                    for j in range(2):
                        off = j * R * F
                        nc.vector.tensor_copy(
                            rep[:, off : off + F], in_t[:, j * F : (j + 1) * F]
                        )
                        w = F
                        while w < R * F:
                            c = min(w, R * F - w)
                            nc.vector.tensor_copy(
                                rep[:, off + w : off + w + c], rep[:, off : off + c]
                            )
                            w += c
                    eng = nc.sync if p % 2 == 0 else nc.scalar
                    eng.dma_start(
                        y.ap()[2 * p].rearrange("(n r) f -> n (r f)", r=R),
                        rep[:, 0 : R * F],
                    )
                    eng.dma_start(
                        y.ap()[2 * p + 1].rearrange("(n r) f -> n (r f)", r=R),
                        rep[:, R * F :],
                    )
    nc.compile()
    return nc


def kernel(x: np.ndarray) -> np.ndarray:
    from concourse.bass_utils import run_bass_kernel_spmd

    x = np.ascontiguousarray(np.asarray(x, dtype=np.float32))
    assert x.shape == (B, N, F), x.shape

    if "nc" not in _CACHE:
        _CACHE["nc"] = _build_nc()
    nc = _CACHE["nc"]

    in_maps = [{"x": x[c * BPC : (c + 1) * BPC]} for c in range(NCORES)]
    res = run_bass_kernel_spmd(nc, in_maps, list(range(NCORES)))
    out = np.concatenate([res.results[c]["y"] for c in range(NCORES)], axis=0)
    return out


# revision 5
# speedup vs baseline: 80.2485x; 1.0839x over previous
"""Trainium2 kernel for nn_NodeEdgeProjection (gnn_message_passing).

Reference computes out = x[:, idx, :] with idx = permutations(range(128), 2)[:, 0]
= [0]*127, [1]*127, ..., i.e. idx[e] = e // 127. So the output is each node row
repeated 127 times along the edge axis — a pure broadcast of [B, N, F] to
[B, N*(N-1), F]. Memory-bound: ~533 MB of output writes.

Sharding: pure data parallel over the batch dim (16 batches per core, 8 cores).

Per-core kernel: nodes live one-per-partition in SBUF. For each pair of
batches, a DVE doubling chain materializes all 127 repeats in a pair tile
(2 x 32.5 KB per partition), then two fully-contiguous 4.16 MB HWDGE DMAs
(one on the SP ring, one on the ACT ring) stream the pair to DRAM. Measured on
HW (marginal over in-NEFF repeats): ~125-145 us/core, DMA-bound at ~500 GB/s —
on par with a pure-DMA lower bound of the same traffic. A stride-0
(broadcast-source) DMA variant was 90x slower on HW despite the cost model
liking it; replicate-in-SBUF + contiguous DMA is the fast path.
"""

import numpy as np

B, N, F = 128, 128, 64
NCORES = 8
BPC = B // NCORES   # batches per core: 16
R = N - 1           # repeats per node: 127

_CACHE = {}


def _build_nc(n_reps: int = 1):
    # n_reps repeats the whole body (same output written each time) — used
    # only by the local timing harness to measure steady-state HW time.
    import concourse.bacc as bacc
    import concourse.mybir as mybir
    import concourse.tile as tile

    fp32 = mybir.dt.float32
    nc = bacc.Bacc("TRN2", target_bir_lowering=False, debug=False)
    x = nc.dram_tensor("x", [BPC, N, F], fp32, kind="ExternalInput")
    y = nc.dram_tensor("y", [BPC, N * R, F], fp32, kind="ExternalOutput")

    with tile.TileContext(nc) as tc:
        with (
            tc.tile_pool(name="inp", bufs=2) as inpool,
            tc.tile_pool(name="rep", bufs=2) as reppool,
        ):
            for _ in range(n_reps):
                for p in range(BPC // 2):
                    # load the pair's two batches: x[b, n, f] -> in_t[n, (b f)]
                    in_t = inpool.tile([N, 2 * F], fp32)
                    nc.sync.dma_start(
                        in_t[:].rearrange("n (b f) -> n b f", b=2),
                        x.ap()[2 * p : 2 * p + 2].rearrange("b n f -> n b f"),
                    )
                    rep = reppool.tile([N, 2 * R * F], fp32)
                    for j in range(2):
                        off = j * R * F
                        nc.vector.tensor_copy(
                            rep[:, off : off + F], in_t[:, j * F : (j + 1) * F]
                        )
                        w = F
                        while w < R * F:
                            c = min(w, R * F - w)
                            nc.vector.tensor_copy(
                                rep[:, off + w : off + w + c], rep[:, off : off + c]
                            )
                            w += c
                    # split the pair across both HWDGE rings (SP + ACT) so each
                    # replication completion feeds both rings at once
                    nc.sync.dma_start(
                        y.ap()[2 * p].rearrange("(n r) f -> n (r f)", r=R),
                        rep[:, 0 : R * F],
                    )
                    nc.scalar.dma_start(
                        y.ap()[2 * p + 1].rearrange("(n r) f -> n (r f)", r=R),
                        rep[:, R * F :],
                    )
    nc.compile()
    return nc


def kernel(x: np.ndarray) -> np.ndarray:
    from concourse.bass_utils import run_bass_kernel_spmd

    x = np.ascontiguousarray(np.asarray(x, dtype=np.float32))
    assert x.shape == (B, N, F), x.shape

    if "nc" not in _CACHE:
        _CACHE["nc"] = _build_nc()
    nc = _CACHE["nc"]

    in_maps = [{"x": x[c * BPC : (c + 1) * BPC]} for c in range(NCORES)]
    res = run_bass_kernel_spmd(nc, in_maps, list(range(NCORES)))
    out = np.concatenate([res.results[c]["y"] for c in range(NCORES)], axis=0)
    return out
